# revision 5
# baseline (speedup 1.0000x reference)
"""ApproxRepSet kernel for 8 TRN2 NeuronCores.

reference:
  t = relu(X @ Wc)            # [B, P, H*E], k = e*H + h
  t = max over e              # [B, P, H]
  t = sum over p              # [B, H]
  t = relu(t @ w1 + b1); t = relu(t @ w2 + b2); out = t @ w3 + b3

Sharding: data-parallel over batch, 16 batches per core. Weights replicated.

Per-core layout (host-side, zero on-device transposes):
  - X shard [16*1024, 64] packed as A[128, 8192]: partition 64*(r%2)+d,
    free r//2.  A 256-row block i lives at free cols [128i, 128i+128): even
    rows on partitions 0:64, odd rows on 64:128.  Each half is the matmul
    stationary lhsT [K=64, M=128]; the two halves run concurrently via PE
    row tiling (tile_position (0,0)/(64,0)).
  - X/Wc cast to bf16 on host; Wc columns reordered k' = h*16 + e so the
    max over e is an innermost free-dim window; Wc stacked twice on
    partitions for row tiling.

Pooling (the throughput wall: every Y element must leave PSUM through DVE
at 0.96 G/lane or ACT at 1.2 G/lane, 1 elem/lane/cycle):
  - Blocks processed in groups of 8 (2 batches), roles [a,b,b,b, a,b,b,b]:
    path a: DVE tensor_reduce(max) straight from PSUM   (~1.2us/blk DVE)
    path b: ACT relu-cast PSUM->SBUF bf16               (~1.07us/blk ACT)
  - The 6 b-blocks of a group share ONE binary TT-max tree (bf16 SBUF runs
    in 2x DVE mode; big trees amortize the ~151c fixed cost per op).  The
    last level is scalar_tensor_tensor (out = (u0 max 0) max u1) fusing the
    relu.  Trees are deferred one group so DVE never waits on ACT.
  - a:b = 16:48 balances DVE (reduces + trees) against ACT (moves).
  - Sum over p: ones-vector matmuls (lhsT = mb [128, 32] slices,
    rhs = ones [128,1]) accumulating S^T [32, 16] in PSUM, issued one group
    late so PE never stalls on the tree.
  - MLP stays transposed end-to-end: h1^T = w1^T @ S^T etc.
"""

import sys

import numpy as np

sys.path.insert(0, "/opt/trn_rl_repo")

import ml_dtypes
import concourse.bass as bass
import concourse.mybir as mybir
import concourse.tile as tile
from concourse import bacc
from concourse.bass_utils import run_bass_kernel_spmd

B, P, D = 128, 1024, 64
H, E = 32, 16
HE = H * E  # 512
NOUT = 10
NCORES = 8
BPC = B // NCORES  # 16 batches per core
R = BPC * P  # 16384 rows per core
NBLK = R // 256  # 64 blocks of 256 rows
NGRP = NBLK // 8  # 8 groups of 8 blocks (2 batches each)
FCHUNK = 2048  # free-dim cols per DMA chunk (= 16 blocks)

FP32 = mybir.dt.float32
BF16 = mybir.dt.bfloat16
AX = mybir.AxisListType
ALU = mybir.AluOpType
ACT_F = mybir.ActivationFunctionType

_cache = {}


def _build_nc():
    nc = bacc.Bacc(
        "TRN2", target_bir_lowering=False, debug=False, num_devices=NCORES
    )

    xa = nc.declare_dram_parameter("xa", [128, R // 2], BF16, isOutput=False)
    wc = nc.declare_dram_parameter("wc", [128, HE], BF16, isOutput=False)
    # packed MLP weights [64, 141] f32: w1 rows 0:32 cols 0:64, w2 cols 64:128,
    # w3 cols 128:138, b1 col 138, b2 col 139, b3 col 140 (rows 0:10)
    wmlp = nc.declare_dram_parameter("wmlp", [64, 141], FP32, isOutput=False)
    out = nc.declare_dram_parameter("out", [NOUT, BPC], FP32, isOutput=True)

    with tile.TileContext(nc) as tc:
        with (
            tc.tile_pool(name="const", bufs=1) as const_pool,
            tc.tile_pool(name="xa", bufs=2) as xa_pool,
            tc.tile_pool(name="mb", bufs=2) as mb_pool,
            tc.tile_pool(name="yb", bufs=2) as yb_pool,
            tc.tile_pool(name="tree", bufs=1) as tree_pool,
            tc.tile_pool(name="mlp", bufs=1) as mlp_pool,
            tc.tile_pool(name="ypsum", bufs=3, space=bass.MemorySpace.PSUM) as ypsum_pool,
            tc.tile_pool(name="spsum", bufs=1, space=bass.MemorySpace.PSUM) as spsum_pool,
        ):
            # --- persistent tiles; first xa chunk + wc first (critical path) ---
            xa_tiles = []
            for c in range(4):
                t = xa_pool.tile([128, FCHUNK], BF16, tag="xa", name="xa_sb")
                xa_tiles.append(t)
            for piece in range(4):
                lo, hi = piece * 512, (piece + 1) * 512
                nc.sync.dma_start(xa_tiles[0][:, lo:hi], xa[:, lo:hi])
            wc_sb = const_pool.tile([128, HE], BF16)
            nc.gpsimd.dma_start(wc_sb[:], wc[:])
            ones_sb = const_pool.tile([128, 1], BF16)
            nc.vector.memset(ones_sb[:], 1.0)

            # one PSUM bank shared by the S^T accumulator and the MLP matmuls
            sm_psum = spsum_pool.tile([64, 512], FP32)
            s_psum = sm_psum[0:H, 0:BPC]  # S^T accumulator

            def do_block(blk, role, mb, aslot, yb, bslot):
                """One 256-row block: 2 row-tiled matmuls + drain (a or b)."""
                xa_sb = xa_tiles[blk // (FCHUNK // 128)]
                f0 = (blk % (FCHUNK // 128)) * 128
                y_ps = ypsum_pool.tile([128, 2 * HE], FP32, tag="y_ps", name="y_ps")
                nc.tensor.matmul(
                    y_ps[:, 0:HE],
                    xa_sb[0:64, f0 : f0 + 128],
                    wc_sb[0:64, :],
                    start=True,
                    stop=True,
                )
                nc.tensor.matmul(
                    y_ps[:, HE : 2 * HE],
                    xa_sb[64:128, f0 : f0 + 128],
                    wc_sb[64:128, :],
                    start=True,
                    stop=True,
                )
                if role == "a":
                    # max over e=16 windows straight out of PSUM (1x DVE)
                    nc.vector.tensor_reduce(
                        mb[:, 2 * aslot : 2 * aslot + 2, :],
                        y_ps[:].rearrange("p (t h e) -> p t h e", t=2, h=H, e=E),
                        axis=AX.X,
                        op=ALU.max,
                    )
                else:
                    # relu-cast to bf16 (1x ACT); tree later
                    nc.scalar.activation(
                        yb[:, 2 * bslot : 2 * bslot + 2, :, :].rearrange(
                            "p a b c -> p (a b c)"
                        ),
                        y_ps[:],
                        ACT_F.Relu,
                    )

            def do_tree(yb, mb):
                """Binary max tree over 6 b-blocks: yb [128, 12, H, 16] ->
                mb[:, 4:16, :], relu fused in the last level."""
                t1 = tree_pool.tile([128, 12, H, 8], BF16, tag="t1", name="t1")
                nc.vector.tensor_tensor(
                    t1[:], yb[:, :, :, 0:8], yb[:, :, :, 8:16], op=ALU.max
                )
                t2 = tree_pool.tile([128, 12, H, 4], BF16, tag="t2", name="t2")
                nc.vector.tensor_tensor(
                    t2[:], t1[:, :, :, 0:4], t1[:, :, :, 4:8], op=ALU.max
                )
                t3 = tree_pool.tile([128, 12, H, 2], BF16, tag="t3", name="t3")
                nc.vector.tensor_tensor(
                    t3[:], t2[:, :, :, 0:2], t2[:, :, :, 2:4], op=ALU.max
                )
                # out = (u0 max 0) max u1 : final pair max + relu in one op
                nc.vector.scalar_tensor_tensor(
                    mb[:, 4:16, :],
                    t3[:, :, :, 0],
                    0.0,
                    t3[:, :, :, 1],
                    op0=ALU.max,
                    op1=ALU.max,
                )

            def finish_group(g, mb):
                """relu the a-slots of group g's mb, then S accumulation."""
                nc.vector.tensor_scalar_max(mb[:, 0:4, :], mb[:, 0:4, :], 0.0)
                # batch A = group blocks 0..3 -> j {0,1} + {4..9}
                # batch B = group blocks 4..7 -> j {2,3} + {10..15}
                for bi, js in ((0, (0, 1, 4, 5, 6, 7, 8, 9)),
                               (1, (2, 3, 10, 11, 12, 13, 14, 15))):
                    bidx = 2 * g + bi
                    for n, j in enumerate(js):
                        nc.tensor.matmul(
                            s_psum[:, bidx : bidx + 1],
                            mb[:, j, :],
                            ones_sb[:],
                            start=(n == 0),
                            stop=(n == 7),
                        )

            prev = None  # (g, yb, mb) of the previous group
            for g in range(NGRP):
                blk0 = 8 * g
                if blk0 % (FCHUNK // 128) == 0 and blk0 > 0:
                    c = blk0 // (FCHUNK // 128)
                    nc.sync.dma_start(
                        xa_tiles[c][:], xa[:, c * FCHUNK : (c + 1) * FCHUNK]
                    )
                mb = mb_pool.tile([128, 16, H], BF16, tag="mb", name="mb")
                yb = yb_pool.tile([128, 12, H, E], BF16, tag="yb", name="yb")
                # roles: [a b b b a b b b]; a-slots 0,1; b-slots 0..5
                do_block(blk0 + 0, "a", mb, 0, yb, 0)
                do_block(blk0 + 1, "b", mb, 0, yb, 0)
                do_block(blk0 + 2, "b", mb, 0, yb, 1)
                do_block(blk0 + 3, "b", mb, 0, yb, 2)
                do_block(blk0 + 4, "a", mb, 1, yb, 0)
                do_block(blk0 + 5, "b", mb, 0, yb, 3)
                do_block(blk0 + 6, "b", mb, 0, yb, 4)
                do_block(blk0 + 7, "b", mb, 0, yb, 5)
                if prev is not None:
                    pg, pyb, pmb = prev
                    do_tree(pyb, pmb)
                    finish_group(pg, pmb)
                prev = (g, yb, mb)

            pg, pyb, pmb = prev
            do_tree(pyb, pmb)
            finish_group(pg, pmb)

            # --- MLP tail (all transposed); weights arrive in one late DMA ---
            wmlp_sb = const_pool.tile([64, 141], FP32)
            nc.gpsimd.dma_start(wmlp_sb[:], wmlp[:])
            w1_sb = wmlp_sb[0:H, 0:64]
            w2_sb = wmlp_sb[0:64, 64:128]
            w3_sb = wmlp_sb[0:64, 128 : 128 + NOUT]
            b1_sb = wmlp_sb[0:64, 138:139]
            b2_sb = wmlp_sb[0:64, 139:140]
            b3_sb = wmlp_sb[0:NOUT, 140:141]

            s_sb = mlp_pool.tile([H, BPC], FP32)
            nc.vector.tensor_copy(s_sb[:], s_psum[:])

            h1_ps = sm_psum[0:64, 64:80]
            nc.tensor.matmul(h1_ps, w1_sb, s_sb[:], start=True, stop=True)
            h1_sb = mlp_pool.tile([64, BPC], FP32)
            nc.scalar.activation(h1_sb[:], h1_ps, ACT_F.Relu, bias=b1_sb)

            h2_ps = sm_psum[0:64, 96:112]
            nc.tensor.matmul(h2_ps, w2_sb, h1_sb[:], start=True, stop=True)
            h2_sb = mlp_pool.tile([64, BPC], FP32)
            nc.scalar.activation(h2_sb[:], h2_ps, ACT_F.Relu, bias=b2_sb)

            o_ps = sm_psum[0:NOUT, 128:144]
            nc.tensor.matmul(o_ps, w3_sb, h2_sb[:], start=True, stop=True)
            o_sb = mlp_pool.tile([NOUT, BPC], FP32)
            nc.scalar.activation(o_sb[:], o_ps, ACT_F.Identity, bias=b3_sb)

            nc.sync.dma_start(out[:], o_sb[:])

    nc.compile()
    return nc


def _prep_shared(Wc, w1, b1, w2, b2, w3, b3):
    # reorder Wc columns: k = e*H + h  ->  k' = h*E + e
    Wc = np.asarray(Wc, dtype=np.float32)
    wc_r = np.ascontiguousarray(
        Wc.reshape(D, E, H).transpose(0, 2, 1).reshape(D, HE)
    )
    wc_stack = np.ascontiguousarray(
        np.concatenate([wc_r, wc_r], axis=0).astype(ml_dtypes.bfloat16)
    )
    wmlp = np.zeros((64, 141), np.float32)
    wmlp[0:H, 0:64] = np.asarray(w1, np.float32)
    wmlp[0:64, 64:128] = np.asarray(w2, np.float32)
    wmlp[0:64, 128 : 128 + NOUT] = np.asarray(w3, np.float32)
    wmlp[0:64, 138] = np.asarray(b1, np.float32)
    wmlp[0:64, 139] = np.asarray(b2, np.float32)
    wmlp[0:NOUT, 140] = np.asarray(b3, np.float32)
    return dict(wc=wc_stack, wmlp=wmlp)


def _pack_x(Xc):
    # Xc [BPC, P, D] -> A [128, R//2]: A[64*(r%2)+d, r//2] = Xc_flat[r, d]
    Xf = np.asarray(Xc, np.float32).reshape(R, D)
    A = Xf.reshape(R // 2, 2, D).transpose(1, 2, 0).reshape(128, R // 2)
    return np.ascontiguousarray(A.astype(ml_dtypes.bfloat16))


def run(X, Wc, w1, b1, w2, b2, w3, b3, trace=False):
    if "nc" not in _cache:
        _cache["nc"] = _build_nc()
    nc = _cache["nc"]

    shared = _prep_shared(Wc, w1, b1, w2, b2, w3, b3)
    in_maps = []
    for c in range(NCORES):
        m = dict(shared)
        m["xa"] = _pack_x(X[c * BPC : (c + 1) * BPC])
        in_maps.append(m)

    res = run_bass_kernel_spmd(
        nc, in_maps, core_ids=list(range(NCORES)), trace=trace
    )
    outs = [np.asarray(r["out"]).T for r in res.results]  # each [BPC, NOUT]
    full = np.concatenate(outs, axis=0).astype(np.float32)
    return full, res


def kernel(X, Wc, w1, b1, w2, b2, w3, b3):
    full, _ = run(X, Wc, w1, b1, w2, b2, w3, b3, trace=False)
    return full


# revision 11
# speedup vs baseline: 1.0604x; 1.0604x over previous
"""ApproxRepSet kernel for 8 TRN2 NeuronCores.

reference:
  t = relu(X @ Wc)            # [B, P, H*E], k = e*H + h
  t = max over e              # [B, P, H]
  t = sum over p              # [B, H]
  t = relu(t @ w1 + b1); t = relu(t @ w2 + b2); out = t @ w3 + b3

Sharding: data-parallel over batch, 16 batches per core. Weights replicated.

Per-core layout (host-side, zero on-device transposes):
  - X shard [16*1024, 64] packed as A[128, 8192]: partition 64*(r%2)+d,
    free r//2.  A 256-row block i lives at free cols [128i, 128i+128): even
    rows on partitions 0:64, odd rows on 64:128.  Each half is the matmul
    stationary lhsT [K=64, M=128]; the two halves run concurrently via PE
    row tiling (tile_position (0,0)/(64,0)).
  - X/Wc cast to bf16 on host; Wc columns reordered k' = h*16 + e so the
    max over e is an innermost free-dim window; Wc stacked twice on
    partitions for row tiling.

Pooling (the throughput wall: every Y element must leave PSUM through DVE
at 0.96 G/lane or ACT at 1.2 G/lane, 1 elem/lane/cycle):
  - Blocks processed in groups of 8 (2 batches), roles [a,b,b,b, a,b,b,b]:
    path a: DVE tensor_reduce(max) straight from PSUM   (~1.2us/blk DVE)
    path b: ACT relu-cast PSUM->SBUF bf16               (~1.07us/blk ACT)
  - The 6 b-blocks of a group share ONE binary TT-max tree (bf16 SBUF runs
    in 2x DVE mode; big trees amortize the ~151c fixed cost per op).  The
    last level is scalar_tensor_tensor (out = (u0 max 0) max u1) fusing the
    relu.  Trees are deferred one group so DVE never waits on ACT.
  - a:b = 16:48 balances DVE (reduces + trees) against ACT (moves).
  - Sum over p: ones-vector matmuls (lhsT = mb [128, 32] slices,
    rhs = ones [128,1]) accumulating S^T [32, 16] in PSUM, issued one group
    late so PE never stalls on the tree.
  - MLP stays transposed end-to-end: h1^T = w1^T @ S^T etc.
"""

import sys

import numpy as np

sys.path.insert(0, "/opt/trn_rl_repo")

import ml_dtypes
import concourse.bass as bass
import concourse.mybir as mybir
import concourse.tile as tile
from concourse import bacc
from concourse.bass_utils import run_bass_kernel_spmd

B, P, D = 128, 1024, 64
H, E = 32, 16
HE = H * E  # 512
NOUT = 10
NCORES = 8
BPC = B // NCORES  # 16 batches per core
R = BPC * P  # 16384 rows per core
NBLK = R // 256  # 64 blocks of 256 rows
NGRP = NBLK // 8  # 8 groups of 8 blocks (2 batches each)
FCHUNK = 2048  # free-dim cols per DMA chunk (= 16 blocks)

FP32 = mybir.dt.float32
BF16 = mybir.dt.bfloat16
AX = mybir.AxisListType
ALU = mybir.AluOpType
ACT_F = mybir.ActivationFunctionType

_cache = {}


def _build_nc():
    nc = bacc.Bacc(
        "TRN2", target_bir_lowering=False, debug=False, num_devices=NCORES
    )

    xa = nc.declare_dram_parameter("xa", [128, R // 2], BF16, isOutput=False)
    wc = nc.declare_dram_parameter("wc", [128, HE], BF16, isOutput=False)
    # packed MLP weights [64, 141] f32: w1 rows 0:32 cols 0:64, w2 cols 64:128,
    # w3 cols 128:138, b1 col 138, b2 col 139, b3 col 140 (rows 0:10)
    wmlp = nc.declare_dram_parameter("wmlp", [64, 141], FP32, isOutput=False)
    out = nc.declare_dram_parameter("out", [NOUT, BPC], FP32, isOutput=True)

    with tile.TileContext(nc) as tc:
        with (
            tc.tile_pool(name="const", bufs=1) as const_pool,
            tc.tile_pool(name="xa", bufs=2) as xa_pool,
            tc.tile_pool(name="mb", bufs=3) as mb_pool,
            tc.tile_pool(name="yb", bufs=2) as yb_pool,
            tc.tile_pool(name="tree", bufs=1) as tree_pool,
            tc.tile_pool(name="mlp", bufs=1) as mlp_pool,
            tc.tile_pool(name="ypsum", bufs=3, space=bass.MemorySpace.PSUM) as ypsum_pool,
            tc.tile_pool(name="spsum", bufs=1, space=bass.MemorySpace.PSUM) as spsum_pool,
        ):
            # --- persistent tiles; wc + first xa chunk first (critical path) ---
            xa_tiles = []
            for c in range(4):
                t = xa_pool.tile([128, FCHUNK], BF16, tag="xa", name="xa_sb")
                xa_tiles.append(t)
            wc_sb = const_pool.tile([128, HE], BF16)
            nc.sync.dma_start(wc_sb[:], wc[:])
            for piece in range(4):
                lo, hi = piece * 512, (piece + 1) * 512
                nc.sync.dma_start(xa_tiles[0][:, lo:hi], xa[:, lo:hi])
            ones_sb = const_pool.tile([128, 1], BF16)
            nc.vector.memset(ones_sb[:], 1.0)
            zeros_sb = const_pool.tile([64, BPC], FP32)
            nc.vector.memset(zeros_sb[:], 0.0)
            # dummy ACTIVATE: hoists the Relu table load into the DMA-wait
            scratch_sb = const_pool.tile([128, 1], BF16)
            nc.scalar.activation(scratch_sb[:], ones_sb[:], ACT_F.Relu)

            # one PSUM bank shared by the S^T accumulator and the MLP matmuls
            sm_psum = spsum_pool.tile([64, 512], FP32)
            s_psum = sm_psum[0:H, 0:BPC]  # S^T accumulator

            def do_block(blk, role, mb, aslot, yb, bslot):
                """One 256-row block: 2 row-tiled matmuls + drain (a or b)."""
                xa_sb = xa_tiles[blk // (FCHUNK // 128)]
                f0 = (blk % (FCHUNK // 128)) * 128
                y_ps = ypsum_pool.tile([128, 2 * HE], FP32, tag="y_ps", name="y_ps")
                nc.tensor.matmul(
                    y_ps[:, 0:HE],
                    xa_sb[0:64, f0 : f0 + 128],
                    wc_sb[0:64, :],
                    start=True,
                    stop=True,
                )
                nc.tensor.matmul(
                    y_ps[:, HE : 2 * HE],
                    xa_sb[64:128, f0 : f0 + 128],
                    wc_sb[64:128, :],
                    start=True,
                    stop=True,
                )
                if role == "a":
                    # max over e=16 windows straight out of PSUM (1x DVE)
                    nc.vector.tensor_reduce(
                        mb[:, 2 * aslot : 2 * aslot + 2, :],
                        y_ps[:].rearrange("p (t h e) -> p t h e", t=2, h=H, e=E),
                        axis=AX.X,
                        op=ALU.max,
                    )
                else:
                    # relu-cast to bf16 (1x ACT); tree later
                    nc.scalar.activation(
                        yb[:, 2 * bslot : 2 * bslot + 2, :, :].rearrange(
                            "p a b c -> p (a b c)"
                        ),
                        y_ps[:],
                        ACT_F.Relu,
                    )

            def do_tree(yb, mb, s0=0, ns=6):
                """Binary max tree over b-slots [s0, s0+ns) of yb
                [128, 12, H, 16] -> mb[:, 4+2*s0 : 4+2*(s0+ns), :],
                relu fused in the last level."""
                q0, q1 = 2 * s0, 2 * (s0 + ns)
                nq = q1 - q0
                t1 = tree_pool.tile([128, nq, H, 8], BF16, tag=f"t1_{nq}", name="t1")
                nc.vector.tensor_tensor(
                    t1[:], yb[:, q0:q1, :, 0:8], yb[:, q0:q1, :, 8:16], op=ALU.max
                )
                t2 = tree_pool.tile([128, nq, H, 4], BF16, tag=f"t2_{nq}", name="t2")
                nc.vector.tensor_tensor(
                    t2[:], t1[:, :, :, 0:4], t1[:, :, :, 4:8], op=ALU.max
                )
                t3 = tree_pool.tile([128, nq, H, 2], BF16, tag=f"t3_{nq}", name="t3")
                nc.vector.tensor_tensor(
                    t3[:], t2[:, :, :, 0:2], t2[:, :, :, 2:4], op=ALU.max
                )
                # out = (u0 max 0) max u1 : final pair max + relu in one op
                nc.vector.scalar_tensor_tensor(
                    mb[:, 4 + q0 : 4 + q1, :],
                    t3[:, :, :, 0],
                    0.0,
                    t3[:, :, :, 1],
                    op0=ALU.max,
                    op1=ALU.max,
                )

            def finish_group(g, mb):
                """S accumulation for group g (mb fully relu'd by now)."""
                # batch A = group blocks 0..3 -> j {0,1} + {4..9}
                # batch B = group blocks 4..7 -> j {2,3} + {10..15}
                for bi, js in ((0, (0, 1, 4, 5, 6, 7, 8, 9)),
                               (1, (2, 3, 10, 11, 12, 13, 14, 15))):
                    bidx = 2 * g + bi
                    for n, j in enumerate(js):
                        nc.tensor.matmul(
                            s_psum[:, bidx : bidx + 1],
                            mb[:, j, :],
                            ones_sb[:],
                            start=(n == 0),
                            stop=(n == 7),
                        )

            tree_pend = []  # groups with pending tree (1 deep)
            ones_pend = []  # groups with pending ones-MMs (2 deep)
            for g in range(NGRP):
                blk0 = 8 * g
                if blk0 % (FCHUNK // 128) == 0 and blk0 > 0:
                    c = blk0 // (FCHUNK // 128)
                    nc.sync.dma_start(
                        xa_tiles[c][:], xa[:, c * FCHUNK : (c + 1) * FCHUNK]
                    )
                mb = mb_pool.tile([128, 16, H], BF16, tag="mb", name="mb")
                yb = yb_pool.tile([128, 12, H, E], BF16, tag="yb", name="yb")
                # roles: [a b b b a b b b]; a-slots 0,1; b-slots 0..5
                do_block(blk0 + 0, "a", mb, 0, yb, 0)
                do_block(blk0 + 1, "b", mb, 0, yb, 0)
                do_block(blk0 + 2, "b", mb, 0, yb, 1)
                do_block(blk0 + 3, "b", mb, 0, yb, 2)
                do_block(blk0 + 4, "a", mb, 1, yb, 0)
                do_block(blk0 + 5, "b", mb, 0, yb, 3)
                do_block(blk0 + 6, "b", mb, 0, yb, 4)
                do_block(blk0 + 7, "b", mb, 0, yb, 5)
                if tree_pend:
                    pg, pyb, pmb = tree_pend.pop(0)
                    do_tree(pyb, pmb)
                    nc.vector.tensor_scalar_max(
                        pmb[:, 0:4, :], pmb[:, 0:4, :], 0.0
                    )
                    ones_pend.append((pg, pmb))
                if len(ones_pend) > 1:
                    og, omb = ones_pend.pop(0)
                    finish_group(og, omb)
                tree_pend.append((g, yb, mb))

            # tail: last group's tree split in two so it overlaps the drains
            pg, pyb, pmb = tree_pend.pop(0)
            do_tree(pyb, pmb, 0, 3)
            do_tree(pyb, pmb, 3, 3)
            nc.vector.tensor_scalar_max(pmb[:, 0:4, :], pmb[:, 0:4, :], 0.0)
            ones_pend.append((pg, pmb))
            for og, omb in ones_pend:
                finish_group(og, omb)

            # --- MLP tail (all transposed); weights arrive in one late DMA ---
            wmlp_sb = const_pool.tile([64, 141], FP32)
            nc.gpsimd.dma_start(wmlp_sb[:], wmlp[:])
            w1_sb = wmlp_sb[0:H, 0:64]
            w2_sb = wmlp_sb[0:64, 64:128]
            w3_sb = wmlp_sb[0:64, 128 : 128 + NOUT]
            b1_sb = wmlp_sb[0:64, 138:139]
            b2_sb = wmlp_sb[0:64, 139:140]
            b3_sb = wmlp_sb[0:NOUT, 140:141]

            s_sb = mlp_pool.tile([H, BPC], FP32)
            nc.vector.tensor_copy(s_sb[:], s_psum[:])

            # relu(x + bias) on DVE: out = (x add bias) max zeros
            h1_ps = sm_psum[0:64, 64:80]
            nc.tensor.matmul(h1_ps, w1_sb, s_sb[:], start=True, stop=True)
            h1_sb = mlp_pool.tile([64, BPC], FP32)
            nc.vector.scalar_tensor_tensor(
                h1_sb[:], h1_ps, b1_sb, zeros_sb[:], op0=ALU.add, op1=ALU.max
            )

            h2_ps = sm_psum[0:64, 96:112]
            nc.tensor.matmul(h2_ps, w2_sb, h1_sb[:], start=True, stop=True)
            h2_sb = mlp_pool.tile([64, BPC], FP32)
            nc.vector.scalar_tensor_tensor(
                h2_sb[:], h2_ps, b2_sb, zeros_sb[:], op0=ALU.add, op1=ALU.max
            )

            o_ps = sm_psum[0:NOUT, 128:144]
            nc.tensor.matmul(o_ps, w3_sb, h2_sb[:], start=True, stop=True)
            o_sb = mlp_pool.tile([NOUT, BPC], FP32)
            nc.vector.scalar_tensor_tensor(
                o_sb[:], o_ps, b3_sb, zeros_sb[0:NOUT, :], op0=ALU.add, op1=ALU.add
            )

            nc.sync.dma_start(out[:], o_sb[:])

    nc.compile()
    return nc


def _prep_shared(Wc, w1, b1, w2, b2, w3, b3):
    # reorder Wc columns: k = e*H + h  ->  k' = h*E + e
    Wc = np.asarray(Wc, dtype=np.float32)
    wc_r = np.ascontiguousarray(
        Wc.reshape(D, E, H).transpose(0, 2, 1).reshape(D, HE)
    )
    wc_stack = np.ascontiguousarray(
        np.concatenate([wc_r, wc_r], axis=0).astype(ml_dtypes.bfloat16)
    )
    wmlp = np.zeros((64, 141), np.float32)
    wmlp[0:H, 0:64] = np.asarray(w1, np.float32)
    wmlp[0:64, 64:128] = np.asarray(w2, np.float32)
    wmlp[0:64, 128 : 128 + NOUT] = np.asarray(w3, np.float32)
    wmlp[0:64, 138] = np.asarray(b1, np.float32)
    wmlp[0:64, 139] = np.asarray(b2, np.float32)
    wmlp[0:NOUT, 140] = np.asarray(b3, np.float32)
    return dict(wc=wc_stack, wmlp=wmlp)


def _pack_x(Xc):
    # Xc [BPC, P, D] -> A [128, R//2]: A[64*(r%2)+d, r//2] = Xc_flat[r, d]
    Xf = np.asarray(Xc, np.float32).reshape(R, D)
    A = Xf.reshape(R // 2, 2, D).transpose(1, 2, 0).reshape(128, R // 2)
    return np.ascontiguousarray(A.astype(ml_dtypes.bfloat16))


def run(X, Wc, w1, b1, w2, b2, w3, b3, trace=False):
    if "nc" not in _cache:
        _cache["nc"] = _build_nc()
    nc = _cache["nc"]

    shared = _prep_shared(Wc, w1, b1, w2, b2, w3, b3)
    in_maps = []
    for c in range(NCORES):
        m = dict(shared)
        m["xa"] = _pack_x(X[c * BPC : (c + 1) * BPC])
        in_maps.append(m)

    res = run_bass_kernel_spmd(
        nc, in_maps, core_ids=list(range(NCORES)), trace=trace
    )
    outs = [np.asarray(r["out"]).T for r in res.results]  # each [BPC, NOUT]
    full = np.concatenate(outs, axis=0).astype(np.float32)
    return full, res


def kernel(X, Wc, w1, b1, w2, b2, w3, b3):
    full, _ = run(X, Wc, w1, b1, w2, b2, w3, b3, trace=False)
    return full
